# revision 1
# baseline (speedup 1.0000x reference)
"""Trainium2 Bass kernel for BugLocalizationGNN (3-layer GAT + classifier).

Sharding: nodes partitioned across 8 cores (6250 dst nodes each); edges
sharded by destination. Per GAT layer:
  1. node-sharded dense matmul h = z @ W (float32r on PE), fused per-head
     attention score columns s = h.a_src, d = h.a_dst via host-precomputed
     [W | W@As | W@Ad] weight blocks
  2. AllGather of the augmented gather table rows [h|1|s] into each core's HBM
  3. per-128-edge-chunk: dma_gather of source rows, one-hot selection matrix
     (DVE iota-compare) matmul-scatter into PSUM accumulating both the
     weighted message sum and the softmax denominator, with edge weights
     w = exp(leakyrelu(s[src]+d[dst])) (global-shift-free softmax — exactly
     equivalent to the segment-max-shifted softmax, values are bounded)
  4. alpha-normalize + (host-folded) BN + ELU on DVE/ACT.

The int16 gather-index limit (< 32768) is handled with two table windows
[0, 32768) and [N-32768, N) selected per chunk at compile time; edges are
split per dst-tile into A/B chunk blocks with uniform (max) chunk counts
across cores so a single SPMD program fits every core.
"""

import numpy as np

P = 128
NCORES = 8
WIN = 32768
PAD_DST = 200.0   # dstcol value for padding lanes (never matches iota 0..127)
PAD_REP = 255     # dstrep value for padding lanes
ECLAMP = 80.0     # safety clamp on attention logits before exp


# ----------------------------------------------------------------------------
# host-side planning
# ----------------------------------------------------------------------------

class Plan:
    pass


def _plan_edges(N, edge_index):
    """Partition edges by dst across cores; build per-core uniform chunk
    structure and the gather-index / selection-matrix input arrays."""
    NL = N // NCORES
    T = (NL + P - 1) // P
    src = np.concatenate([edge_index[0].astype(np.int64), np.arange(N, dtype=np.int64)])
    dst = np.concatenate([edge_index[1].astype(np.int64), np.arange(N, dtype=np.int64)])

    winb_base = N - WIN if N > WIN else None

    # bucket edges per (core, tile), split by src window
    tiles_a = [[None] * T for _ in range(NCORES)]
    tiles_b = [[None] * T for _ in range(NCORES)]
    core_of = dst // NL
    dloc = dst - core_of * NL
    tile_of = dloc // P
    lane_of = dloc - tile_of * P
    for k in range(NCORES):
        mk = core_of == k
        sk, tk, lk = src[mk], tile_of[mk], lane_of[mk]
        for t in range(T):
            mt = tk == t
            s_t, l_t = sk[mt], lk[mt]
            order = np.argsort(s_t, kind="stable")
            s_t, l_t = s_t[order], l_t[order]
            if winb_base is None:
                ma = np.ones(len(s_t), bool)
            else:
                ma = s_t < WIN
            tiles_a[k][t] = (s_t[ma], l_t[ma])
            tiles_b[k][t] = (s_t[~ma] - (winb_base or 0), l_t[~ma])

    cdiv = lambda a, b: -(-a // b)
    CH_A = max(max(cdiv(len(tiles_a[k][t][0]), P), 1) for k in range(NCORES) for t in range(T))
    CH_B = max(cdiv(len(tiles_b[k][t][0]), P) for k in range(NCORES) for t in range(T))

    # group tiles in pairs; chunk sequence per group: A-run (t0 A-chunks, t1
    # A-chunks) then B-run.  Blocks of <=8 chunks per dma_gather instruction.
    groups = [tuple(range(g, min(g + 2, T))) for g in range(0, T, 2)]
    K_CH = CH_A + CH_B
    NCHUNK = T * K_CH
    E_pad = NCHUNK * P

    # compile-time metadata shared by all cores
    chunk_meta = []   # per chunk: (tile, first, last)
    blocks = []       # flat list per dma_gather: (win, chunk0, nchunks)
    grp_meta = []     # per group: dict(c0, nch, runs=[(win, c0, nch, blocks)])
    counts = {t: 0 for t in range(T)}
    total = {t: (CH_A + CH_B) for t in range(T)}
    gc = 0
    for grp in groups:
        gm = dict(grp=grp, c0=gc, runs=[])
        for win, chw in (("A", CH_A), ("B", CH_B)):
            if chw == 0:
                continue
            nch = chw * len(grp)
            rblocks = []
            for b0 in range(0, nch, 8):
                blk = (win, gc + b0, min(8, nch - b0))
                rblocks.append(blk)
                blocks.append(blk)
            gm["runs"].append((win, gc, nch, rblocks))
            for t in grp:
                for _ in range(chw):
                    c = counts[t]
                    chunk_meta.append((t, c == 0, c == total[t] - 1))
                    counts[t] += 1
                    gc += 1
        gm["nch"] = gc - gm["c0"]
        grp_meta.append(gm)
    assert gc == NCHUNK

    # per-core arrays
    idx_cols = E_pad // 16
    idx16 = np.zeros((NCORES, P, idx_cols), np.int16)
    dstcol = np.full((NCORES, P, NCHUNK), PAD_DST, np.float32)
    dstrep = np.full((NCORES, P, E_pad), PAD_REP, np.uint8)

    for k in range(NCORES):
        flat_idx = np.zeros(E_pad, np.int16)
        flat_lane = np.full(E_pad, -1, np.int64)
        gc = 0
        for grp in groups:
            for win, chw in (("A", CH_A), ("B", CH_B)):
                if chw == 0:
                    continue
                for t in grp:
                    s_t, l_t = (tiles_a if win == "A" else tiles_b)[k][t]
                    n = len(s_t)
                    o = gc * P
                    flat_idx[o:o + n] = s_t.astype(np.int16)
                    flat_lane[o:o + n] = l_t
                    gc += chw
        # wrapped+replicated index layout per gather block
        for win, c0, nch in blocks:
            seg = flat_idx[c0 * P:(c0 + nch) * P]
            wrapped = seg.reshape(-1, 16).T            # [16, n/16]
            col0 = c0 * P // 16
            idx16[k, :, col0:col0 + wrapped.shape[1]] = np.tile(wrapped, (8, 1))
        lane = flat_lane.reshape(NCHUNK, P).T          # [P, NCHUNK]
        valid = lane >= 0
        dstcol[k][valid] = lane[valid].astype(np.float32)
        rep = np.where(flat_lane >= 0, flat_lane, PAD_REP).astype(np.uint8)
        dstrep[k] = np.tile(rep[None, :], (P, 1))

    pl = Plan()
    pl.N, pl.NL, pl.T = N, NL, T
    pl.CH_A, pl.CH_B, pl.K_CH = CH_A, CH_B, K_CH
    pl.NCHUNK, pl.E_pad = NCHUNK, E_pad
    pl.groups, pl.chunk_meta, pl.blocks = groups, chunk_meta, blocks
    pl.grp_meta = grp_meta
    pl.winb_base = winb_base
    pl.idx16, pl.dstcol, pl.dstrep = idx16, dstcol, dstrep
    return pl


def _fold_bn(g, be, rm, rv, b, eps=1e-5):
    k = (g / np.sqrt(rv + eps)).astype(np.float64)
    c = (b.astype(np.float64) - rm) * k + be
    return k.astype(np.float32), c.astype(np.float32)


def _prep_weights(W, a_s, a_d, bias, g, be, rm, rv):
    """Host precompute: [Wmain | Wsd] blocks and folded BN constants."""
    IN = W.shape[0]
    Hh, C = a_s.shape
    Wmain = W.astype(np.float32)                      # [IN, H*C]
    Ws = np.zeros((IN, Hh), np.float32)
    Wd = np.zeros((IN, Hh), np.float32)
    for h in range(Hh):
        blk = W[:, h * C:(h + 1) * C].astype(np.float64)
        Ws[:, h] = (blk @ a_s[h].astype(np.float64)).astype(np.float32)
        Wd[:, h] = (blk @ a_d[h].astype(np.float64)).astype(np.float32)
    Wsd = np.concatenate([Ws, Wd], axis=1)            # [IN, 2H]
    k, c = _fold_bn(np.asarray(g, np.float64), np.asarray(be, np.float64),
                    np.asarray(rm, np.float64), np.asarray(rv, np.float64),
                    np.asarray(bias, np.float64))
    return Wmain, Wsd, np.tile(k, (P, 1)), np.tile(c, (P, 1))


# ----------------------------------------------------------------------------
# device program
# ----------------------------------------------------------------------------

def _build_program(pl, dims):
    import concourse.tile as tile
    from concourse import bacc, mybir

    f32 = mybir.dt.float32
    f32r = mybir.dt.float32r
    i16 = mybir.dt.int16
    u8 = mybir.dt.uint8
    AF = mybir.ActivationFunctionType
    OP = mybir.AluOpType

    NL, T = pl.NL, pl.T
    layers = dims["layers"]   # list of dicts: IN, H, C, ROWW, AUGW
    HID = dims["HID"]

    nc = bacc.Bacc("TRN2", target_bir_lowering=False, debug=False,
                   num_devices=NCORES)

    def din(name, shape, dt=f32):
        return nc.dram_tensor(name, list(shape), dt, kind="ExternalInput").ap()

    x_fm = din("x_fm", (layers[0]["IN"], NL))
    eidx = din("eidx", pl.idx16.shape[1:], i16)
    dstcol = din("dstcol", pl.dstcol.shape[1:])
    dstrep_d = din("dstrep", pl.dstrep.shape[1:], u8)
    iota_row_d = din("iota_row", (P, P))
    iota_col_d = din("iota_col", (P, 1))
    Wmain_d, Wsd_d, krep_d, crep_d = [], [], [], []
    for li, L in enumerate(layers):
        Wmain_d.append(din(f"Wmain{li}", (L["IN"], L["H"] * L["C"])))
        Wsd_d.append(din(f"Wsd{li}", (L["IN"], 2 * L["H"])))
        krep_d.append(din(f"krep{li}", (P, L["H"] * L["C"] if L["concat"] else L["C"])))
        crep_d.append(din(f"crep{li}", (P, L["H"] * L["C"] if L["concat"] else L["C"])))
    Wc_d = din("Wc", (HID, 2))
    bcrep_d = din("bcrep", (P, 2))

    out_d = nc.dram_tensor("out", [NL, 2], f32, kind="ExternalOutput").ap()
    dbg = dims.get("debug", False)
    dbg_d = {}
    if dbg:
        for li, L in enumerate(layers):
            dbg_d[f"dbg_haug{li}"] = nc.dram_tensor(
                f"dbg_haug{li}", [NL, L["ROWW"]], f32, kind="ExternalOutput").ap()
            dbg_d[f"dbg_dloc{li}"] = nc.dram_tensor(
                f"dbg_dloc{li}", [P, (NL + P - 1) // P * L["H"]], f32,
                kind="ExternalOutput").ap()
            if li + 1 < len(layers):
                dbg_d[f"dbg_zfm{li}"] = nc.dram_tensor(
                    f"dbg_zfm{li}", [layers[li + 1]["IN"], NL], f32,
                    kind="ExternalOutput").ap()

    # internal DRAM
    haug_loc, haug_full, zfm = [], [], []
    for li, L in enumerate(layers):
        haug_loc.append(nc.dram_tensor(f"haug_loc{li}", [NL, L["ROWW"]], f32).ap())
        haug_full.append(nc.dram_tensor(f"haug_full{li}", [pl.N, L["ROWW"]], f32,
                                        addr_space="Shared").ap())
        if li + 1 < len(layers):
            zfm.append(nc.dram_tensor(f"zfm{li}", [layers[li + 1]["IN"], NL], f32).ap())

    with tile.TileContext(nc) as tc:
        _emit(tc, nc, pl, dims, locals(), mybir)
    nc.compile()
    return nc


def _emit(tc, nc, pl, dims, refs, mybir):
    from contextlib import ExitStack
    from concourse.masks import make_identity

    f32 = mybir.dt.float32
    f32r = mybir.dt.float32r
    i16 = mybir.dt.int16
    u8 = mybir.dt.uint8
    AF = mybir.ActivationFunctionType
    OP = mybir.AluOpType

    NL, T, N = pl.NL, pl.T, pl.N
    layers = dims["layers"]
    HID = dims["HID"]
    x_fm, eidx, dstcol, dstrep_d = refs["x_fm"], refs["eidx"], refs["dstcol"], refs["dstrep_d"]
    iota_row_d, iota_col_d = refs["iota_row_d"], refs["iota_col_d"]
    Wmain_d, Wsd_d, krep_d, crep_d = refs["Wmain_d"], refs["Wsd_d"], refs["krep_d"], refs["crep_d"]
    Wc_d, bcrep_d, out_d = refs["Wc_d"], refs["bcrep_d"], refs["out_d"]
    haug_loc, haug_full, zfm = refs["haug_loc"], refs["haug_full"], refs["zfm"]

    ctx = ExitStack()
    with ctx:
        const = ctx.enter_context(tc.tile_pool(name="const", bufs=1))
        wpool = ctx.enter_context(tc.tile_pool(name="wpool", bufs=1))
        mm_in = ctx.enter_context(tc.tile_pool(name="mm_in", bufs=3))
        aug_pool = ctx.enter_context(tc.tile_pool(name="aug", bufs=3))
        gpool = ctx.enter_context(tc.tile_pool(name="gpool", bufs=3))
        rep_pool = ctx.enter_context(tc.tile_pool(name="rep", bufs=2))
        sel_pool = ctx.enter_context(tc.tile_pool(name="sel", bufs=4))
        wg_pool = ctx.enter_context(tc.tile_pool(name="wg", bufs=3))
        ev_pool = ctx.enter_context(tc.tile_pool(name="ev", bufs=2))
        post_pool = ctx.enter_context(tc.tile_pool(name="post", bufs=3))
        keep = ctx.enter_context(tc.tile_pool(name="keep", bufs=1))

        # ---- resident constants
        iota_row = const.tile([P, P], f32)
        nc.sync.dma_start(out=iota_row[:], in_=iota_row_d[:])
        iota_col = const.tile([P, 1], f32)
        nc.sync.dma_start(out=iota_col[:], in_=iota_col_d[:])
        ident_f = const.tile([P, P], f32)
        make_identity(nc, ident_f[:])
        ident = const.tile([P, P], f32r)
        nc.vector.tensor_copy(ident[:], ident_f[:])
        idx_sb = const.tile(list(pl.idx16.shape[1:]), i16)
        nc.sync.dma_start(out=idx_sb[:], in_=eidx[:])
        dstcol_sb = const.tile(list(pl.dstcol.shape[1:]), f32)
        nc.sync.dma_start(out=dstcol_sb[:], in_=dstcol[:])

        Wmain_sb, Wsd_sb, krep_sb, crep_sb = [], [], [], []
        for li, L in enumerate(layers):
            wm = wpool.tile([P, L["IN"] // P, L["H"] * L["C"]], f32r, tag=f"wm{li}")
            nc.gpsimd.dma_start(
                out=wm[:],
                in_=Wmain_d[li][:].rearrange("(a p) n -> p a n", p=P))
            Wmain_sb.append(wm)
            ws = wpool.tile([P, L["IN"] // P, 2 * L["H"]], f32r, tag=f"ws{li}")
            nc.gpsimd.dma_start(
                out=ws[:],
                in_=Wsd_d[li][:].rearrange("(a p) n -> p a n", p=P))
            Wsd_sb.append(ws)
            FW = L["H"] * L["C"] if L["concat"] else L["C"]
            kt = wpool.tile([P, FW], f32, tag=f"k{li}")
            nc.sync.dma_start(out=kt[:], in_=krep_d[li][:])
            krep_sb.append(kt)
            ct = wpool.tile([P, FW], f32, tag=f"c{li}")
            nc.sync.dma_start(out=ct[:], in_=crep_d[li][:])
            crep_sb.append(ct)
        Wc_sb = wpool.tile([P, 2], f32r)
        nc.gpsimd.dma_start(out=Wc_sb[:], in_=Wc_d[:])
        bcrep_sb = wpool.tile([P, 2], f32)
        nc.sync.dma_start(out=bcrep_sb[:], in_=bcrep_d[:])

        d_loc = [keep.tile([P, T * L["H"]], f32, tag=f"dloc{li}",
                           name=f"dloc{li}")
                 for li, L in enumerate(layers)]
        for dl in d_loc:
            nc.vector.memset(dl[:], 0.0)

        def rows_of(t):
            return min(P, NL - t * P)

        # ------------------------------------------------------------------
        def matmul_phase(li, mm_ps, mm_sd_ps):
            L = layers[li]
            H, C, IN, ROWW = L["H"], L["C"], L["IN"], L["ROWW"]
            NF = H * C
            KT = IN // P
            zin = x_fm if li == 0 else zfm[li - 1]
            for t in range(T):
                mt = rows_of(t)
                lhs = mm_in.tile([P, KT, P], f32r, tag="lhs")
                nc.gpsimd.dma_start(
                    out=lhs[:, :, :mt],
                    in_=zin[:].rearrange("(a p) n -> p a n", p=P)[:, :, t * P:t * P + mt])
                ps1 = mm_ps.tile([P, NF], f32)
                ps2 = mm_sd_ps.tile([P, 2 * H], f32)
                for kk in range(KT):
                    nc.tensor.matmul(out=ps1[:mt, :], lhsT=lhs[:, kk, :mt],
                                     rhs=Wmain_sb[li][:, kk, :],
                                     start=(kk == 0), stop=(kk == KT - 1))
                    nc.tensor.matmul(out=ps2[:mt, :], lhsT=lhs[:, kk, :mt],
                                     rhs=Wsd_sb[li][:, kk, :],
                                     start=(kk == 0), stop=(kk == KT - 1))
                aug = aug_pool.tile([P, ROWW], f32, tag="aug")
                a3 = aug[:, :H * (C + 1)].rearrange("p (h c) -> p h c", h=H)
                nc.vector.tensor_copy(
                    out=a3[:mt, :, :C],
                    in_=ps1[:mt, :].rearrange("p (h c) -> p h c", h=H))
                nc.vector.memset(a3[:mt, :, C:C + 1], 1.0)
                nc.vector.tensor_copy(out=aug[:mt, H * (C + 1):H * (C + 2)],
                                      in_=ps2[:mt, :H])
                nc.vector.tensor_copy(
                    out=d_loc[li][:mt, t * H:(t + 1) * H],
                    in_=ps2[:mt, H:2 * H])
                nc.sync.dma_start(out=haug_loc[li][t * P:t * P + mt, :],
                                  in_=aug[:mt, :])

            if dims.get("nocc"):
                nc.sync.dma_start(out=haug_full[li][:NL, :], in_=haug_loc[li][:])
            else:
                nc.gpsimd.collective_compute(
                    "AllGather", mybir.AluOpType.bypass,
                    replica_groups=[list(range(NCORES))],
                    ins=[haug_loc[li][:].opt()],
                    outs=[haug_full[li][:].opt()],
                )

        # ------------------------------------------------------------------
        def agg_phase(li, agg_ps, den_ps, dexp_ps, tr_ps):
            L = layers[li]
            H, C, ROWW = L["H"], L["C"], L["ROWW"]
            CP1 = C + 1
            for gm in pl.grp_meta:
                grp = gm["grp"]
                ps_main = {}
                for t in grp:
                    ps_main[t] = agg_ps.tile([P, H * C + (4 if H == 1 else 0)],
                                             f32, tag="agm", name=f"agm{t}")
                if H > 1:
                    ps_den = {t: den_ps.tile([P, H], f32, tag="den",
                                             name=f"den{t}")[:]
                              for t in grp}
                g_c0, g_nch = gm["c0"], gm["nch"]
                rep_sb = rep_pool.tile([P, g_nch * P], u8, tag="rep")
                nc.sync.dma_start(out=rep_sb[:],
                                  in_=dstrep_d[:, g_c0 * P:(g_c0 + g_nch) * P])

                for win, c0, nch, rblocks in gm["runs"]:
                    # gathers for this run
                    gtiles = []
                    base = 0 if win == "A" else pl.winb_base
                    for bwin, bc0, bn in rblocks:
                        gt = gpool.tile([P, bn, ROWW], f32, tag="G")
                        if "gather" in AB:
                            gtiles.append((bc0, bn, gt)); continue
                        nc.gpsimd.dma_gather(
                            out_ap=gt[:],
                            in_ap=haug_full[li][base:base + min(WIN, N), :],
                            idxs_ap=idx_sb[:, bc0 * P // 16:(bc0 + bn) * P // 16],
                            num_idxs=bn * P, num_idxs_reg=bn * P,
                            elem_size=ROWW)
                        gtiles.append((bc0, bn, gt))
                    # d_exp for the run
                    psd = dexp_ps.tile([P, nch * H], f32, tag="dexp")
                    for ci in range(nch):
                        if "dexp" in AB:
                            break
                        gc = c0 + ci
                        t = pl.chunk_meta[gc][0]
                        selT = sel_pool.tile([P, P], f32, tag="selT")
                        nc.vector.tensor_scalar(
                            out=selT[:], in0=rep_sb[:, (gc - g_c0) * P:(gc - g_c0 + 1) * P],
                            scalar1=iota_col[:], scalar2=None, op0=OP.is_equal)
                        nc.tensor.matmul(out=psd[:, ci * H:(ci + 1) * H],
                                         lhsT=selT[:],
                                         rhs=d_loc[li][:, t * H:(t + 1) * H],
                                         start=True, stop=True)
                    # batched e-values for the run
                    ev = ev_pool.tile([P, nch * H], f32, tag="ev")
                    sv = ev_pool.tile([P, nch * H], f32, tag="sv")
                    for (bc0, bn, gt) in gtiles:
                        nc.vector.tensor_copy(
                            out=sv[:, (bc0 - c0) * H:(bc0 - c0 + bn) * H]
                                .rearrange("p (b h) -> p b h", h=H),
                            in_=gt[:, :, H * CP1:H * CP1 + H])
                    nc.vector.tensor_add(ev[:], sv[:], psd[:])
                    nc.vector.scalar_tensor_tensor(
                        out=ev[:], in0=ev[:], scalar=0.2, op0=OP.mult,
                        op1=OP.max, in1=ev[:])
                    nc.vector.tensor_scalar(out=ev[:], in0=ev[:], scalar1=ECLAMP,
                                            scalar2=None, op0=OP.min)
                    nc.scalar.activation(out=ev[:], in_=ev[:], func=AF.Exp)
                    # weighted scatter matmuls
                    for (bc0, bn, gt) in gtiles:
                        for j in range(bn):
                            gc = bc0 + j
                            ci = gc - c0
                            t, first, last = pl.chunk_meta[gc]
                            sel = sel_pool.tile([P, P], f32r, tag="sel")
                            if "selbuild" not in AB:
                                nc.vector.tensor_scalar(
                                    out=sel[:], in0=iota_row[:],
                                    scalar1=dstcol_sb[:, gc:gc + 1],
                                    scalar2=None, op0=OP.is_equal)
                            CW = CP1 if H > 1 else CP1 + 3
                            wg = wg_pool.tile([P, H, CW], f32r, tag="wg")
                            if "wg" in AB:
                                nc.vector.memset(wg[:, 0, 0:1].bitcast(f32), 1.0)
                            else:
                                nc.vector.tensor_tensor(
                                out=wg[:],
                                in0=gt[:, j, :H * CW].rearrange("p (h c) -> p h c", h=H),
                                in1=ev[:, ci * H:(ci + 1) * H]
                                    .rearrange("p (h c) -> p h c", c=1)
                                    .to_broadcast([P, H, CW]),
                                op=OP.mult)
                            if "aggmm" in AB:
                                continue
                            if H > 1:
                                nc.tensor.matmul(
                                    out=ps_main[t][:].rearrange("p (h c) -> p h c", h=H),
                                    lhsT=sel[:], rhs=wg[:, :, :C],
                                    start=first, stop=last)
                                nc.tensor.matmul(
                                    out=ps_den[t], lhsT=sel[:],
                                    rhs=wg[:, :, C:CP1].rearrange("p h c -> p (h c)"),
                                    start=first, stop=last)
                            else:
                                nc.tensor.matmul(
                                    out=ps_main[t][:], lhsT=sel[:],
                                    rhs=wg[:, 0, :],
                                    start=first, stop=last)
                # ---- post-processing for the group's tiles
                for t in grp:
                    mt = rows_of(t)
                    FW = H * C if L["concat"] else C
                    rc = post_pool.tile([P, H], f32, tag="rc")
                    if H > 1:
                        nc.vector.reciprocal(rc[:], ps_den[t])
                    else:
                        nc.vector.reciprocal(rc[:], ps_main[t][:, C:C + 1])
                    zt = post_pool.tile([P, FW], f32, tag="zt")
                    if L["concat"]:
                        nc.vector.tensor_tensor(
                            out=zt[:].rearrange("p (h c) -> p h c", h=H),
                            in0=ps_main[t][:].rearrange("p (h c) -> p h c", h=H),
                            in1=rc[:].rearrange("p (h c) -> p h c", c=1)
                                .to_broadcast([P, H, C]),
                            op=OP.mult)
                    else:
                        # H==1 mean over heads is identity
                        nc.vector.tensor_tensor(
                            out=zt[:], in0=ps_main[t][:, :C],
                            in1=rc[:, 0:1].to_broadcast([P, C]), op=OP.mult)
                    nc.vector.tensor_tensor(out=zt[:], in0=zt[:], in1=krep_sb[li][:],
                                            op=OP.mult)
                    nc.vector.tensor_tensor(out=zt[:], in0=zt[:], in1=crep_sb[li][:],
                                            op=OP.add)
                    mneg = post_pool.tile([P, FW], f32, tag="mneg")
                    nc.vector.tensor_scalar(out=mneg[:], in0=zt[:], scalar1=0.0,
                                            scalar2=None, op0=OP.min)
                    nc.scalar.activation(out=mneg[:], in_=mneg[:], func=AF.Exp)
                    zf = post_pool.tile([P, FW], f32r, tag="zf")
                    nc.vector.scalar_tensor_tensor(
                        out=zf[:], in0=mneg[:], scalar=-1.0,
                        op0=OP.add, op1=OP.max, in1=zt[:])
                    if li + 1 < len(layers):
                        # transpose to feature-major for the next matmul phase
                        for h in range(FW // P):
                            pt = tr_ps.tile([P, P], f32r, tag="tr")
                            nc.tensor.matmul(out=pt[:], lhsT=zf[:, h * P:(h + 1) * P],
                                             rhs=ident[:], is_transpose=True,
                                             start=True, stop=True)
                            zc = post_pool.tile([P, P], f32, tag="zc")
                            nc.vector.tensor_copy(zc[:], pt[:])
                            nc.sync.dma_start(
                                out=zfm[li][h * P:(h + 1) * P, t * P:t * P + mt],
                                in_=zc[:, :mt])
                    else:
                        # classifier
                        pt = tr_ps.tile([P, P], f32r, tag="tr")
                        nc.tensor.matmul(out=pt[:], lhsT=zf[:, :P], rhs=ident[:],
                                         is_transpose=True, start=True, stop=True)
                        zc = post_pool.tile([P, P], f32r, tag="zcr")
                        nc.vector.tensor_copy(zc[:], pt[:])
                        pc = den_ps.tile([P, 2], f32, tag="pc")
                        nc.tensor.matmul(out=pc[:mt, :], lhsT=zc[:, :mt], rhs=Wc_sb[:],
                                         start=True, stop=True)
                        ot = post_pool.tile([P, 2], f32, tag="ot")
                        nc.vector.tensor_tensor(out=ot[:mt, :], in0=pc[:mt, :],
                                                in1=bcrep_sb[:mt, :], op=OP.add)
                        nc.sync.dma_start(out=out_d[t * P:t * P + mt, :],
                                          in_=ot[:mt, :])

        AB = dims.get("ablate", set())
        dbg_d = refs.get("dbg_d", {})
        for _rep in range(dims.get("reps", 1)):
          for li in range(len(layers)):
            with tc.tile_pool(name=f"mm_ps{li}", bufs=2, space="PSUM") as mm_ps, \
                 tc.tile_pool(name=f"mm_sd_ps{li}", bufs=2, space="PSUM") as mm_sd_ps:
                matmul_phase(li, mm_ps, mm_sd_ps)
            if dbg_d:
                nc.sync.dma_start(out=dbg_d[f"dbg_haug{li}"][:], in_=haug_loc[li][:])
                nc.sync.dma_start(out=dbg_d[f"dbg_dloc{li}"][:],
                                  in_=d_loc[li][:].bitcast(f32))
            with tc.tile_pool(name=f"agg_ps{li}", bufs=2, space="PSUM") as agg_ps, \
                 tc.tile_pool(name=f"den_ps{li}", bufs=2, space="PSUM") as den_ps, \
                 tc.tile_pool(name=f"dexp_ps{li}", bufs=1, space="PSUM") as dexp_ps, \
                 tc.tile_pool(name=f"tr_ps{li}", bufs=2, space="PSUM") as tr_ps:
                agg_phase(li, agg_ps, den_ps, dexp_ps, tr_ps)
            if dbg_d and li + 1 < len(layers):
                nc.sync.dma_start(out=dbg_d[f"dbg_zfm{li}"][:], in_=zfm[li][:])


# ----------------------------------------------------------------------------
# entry point
# ----------------------------------------------------------------------------

def _layer_dims(IN, H, C, concat):
    NF = H * C
    used = H * (C + 1) + H          # features+ones | s columns
    roww = -(-used * 4 // 256) * 64  # pad row to multiple of 256 bytes (f32)
    return dict(IN=IN, H=H, C=C, concat=concat, ROWW=roww, AUGW=used)


def build_all(x, edge_index, W1, a1s, a1d, b1, g1, be1, rm1, rv1,
              W2, a2s, a2d, b2, g2, be2, rm2, rv2,
              W3, a3s, a3d, b3, g3, be3, rm3, rv3, Wc, bc, debug=False,
              nocc=False, ablate=(), reps=1):
    x = np.asarray(x)
    N, IN = x.shape
    HID = W3.shape[1]
    H = a1s.shape[0]
    pl = _plan_edges(N, np.asarray(edge_index))
    layers = [
        _layer_dims(IN, H, W1.shape[1] // H, True),
        _layer_dims(W1.shape[1], H, W2.shape[1] // H, True),
        _layer_dims(W2.shape[1], 1, W3.shape[1], False),
    ]
    dims = dict(layers=layers, HID=HID, debug=debug, nocc=nocc,
                ablate=set(ablate), reps=reps)

    Wm1, Wsd1, k1, c1 = _prep_weights(W1, a1s, a1d, b1, g1, be1, rm1, rv1)
    Wm2, Wsd2, k2, c2 = _prep_weights(W2, a2s, a2d, b2, g2, be2, rm2, rv2)
    Wm3, Wsd3, k3, c3 = _prep_weights(W3, a3s, a3d, b3, g3, be3, rm3, rv3)

    iota_row = np.tile(np.arange(P, dtype=np.float32), (P, 1))
    iota_col = np.arange(P, dtype=np.float32).reshape(P, 1)

    in_maps = []
    for k in range(NCORES):
        m = dict(
            x_fm=np.ascontiguousarray(x[k * pl.NL:(k + 1) * pl.NL].T),
            eidx=pl.idx16[k], dstcol=pl.dstcol[k], dstrep=pl.dstrep[k],
            iota_row=iota_row, iota_col=iota_col,
            Wmain0=Wm1, Wsd0=Wsd1, krep0=k1, crep0=c1,
            Wmain1=Wm2, Wsd1=Wsd2, krep1=k2, crep1=c2,
            Wmain2=Wm3, Wsd2=Wsd3, krep2=k3, crep2=c3,
            Wc=np.asarray(Wc, np.float32),
            bcrep=np.tile(np.asarray(bc, np.float32), (P, 1)),
        )
        in_maps.append(m)

    nc = _build_program(pl, dims)
    return nc, in_maps, pl


def kernel(**inputs):
    from concourse.bass_utils import run_bass_kernel_spmd
    nc, in_maps, pl = build_all(**inputs)
    res = run_bass_kernel_spmd(nc, in_maps, core_ids=list(range(NCORES)))
    out = np.concatenate([res.results[k]["out"] for k in range(NCORES)], axis=0)
    return out.astype(np.float32)



# revision 43
# speedup vs baseline: 15.1953x; 15.1953x over previous
"""Trainium2 Bass kernel for BugLocalizationGNN (3-layer GAT + classifier).

Sharding: nodes partitioned across 8 cores (6250 dst nodes each); edges
sharded by destination. Per GAT layer:
  1. node-sharded dense matmul h = z @ W (bf16 on PE), fused per-head
     attention score columns s = h.a_src, d = h.a_dst via host-precomputed
     [W | W@As | W@Ad] weight blocks
  2. AllGather of the augmented gather table rows [h|1|s] (bf16) into each
     core's HBM.  The table is split into two sub-tables A (first 4096
     rows/core, 8*4096 = 32768 global rows) and B (remaining 2154 rows/core,
     17232 global rows) so that (a) both tables are addressable with int16
     gather indices and (b) the two AllGathers can be issued separately:
     AG_A fires as soon as the first 32 row-tiles are computed and overlaps
     the rest of the matmul phase; edge chunks whose sources live in table A
     only wait on AG_A.
  3. per-128-edge-chunk: dma_gather of source rows, one-hot selection matrix
     (DVE iota-compare, bf16) matmul-scatter into PSUM accumulating both the
     weighted message sum and the softmax denominator, with edge weights
     w = exp(leakyrelu(s[src]+d[dst])) (global-shift-free softmax — exactly
     equivalent to the segment-max-shifted softmax, values are bounded)
  4. alpha-normalize + (host-folded) BN + ELU on DVE/ACT.
"""

import numpy as np

P = 128
NCORES = 8
G_A = 4096                # rows per core in sub-table A (8*4096 = 32768)
PAD_DST = 200.0           # dstcol value for padding lanes (never matches iota)
PAD_REP = 255             # dstrep value for padding lanes
ECLAMP = 80.0             # safety clamp on attention logits before exp


# ----------------------------------------------------------------------------
# host-side planning
# ----------------------------------------------------------------------------

class Plan:
    pass


def _plan_edges(N, edge_index):
    """Partition edges by dst across cores; build per-core uniform chunk
    structure and the gather-index / selection-matrix input arrays."""
    NL = N // NCORES
    G_B = NL - G_A
    T = (NL + P - 1) // P
    src = np.concatenate([edge_index[0].astype(np.int64), np.arange(N, dtype=np.int64)])
    dst = np.concatenate([edge_index[1].astype(np.int64), np.arange(N, dtype=np.int64)])

    # map src node -> (window, table-local index)
    core_s = src // NL
    r_s = src - core_s * NL
    in_a = r_s < G_A
    tidx = np.where(in_a, core_s * G_A + r_s, core_s * G_B + (r_s - G_A))

    # bucket edges per (core, tile), split by src window
    tiles_a = [[None] * T for _ in range(NCORES)]
    tiles_b = [[None] * T for _ in range(NCORES)]
    core_of = dst // NL
    dloc = dst - core_of * NL
    tile_of = dloc // P
    lane_of = dloc - tile_of * P
    for k in range(NCORES):
        mk = core_of == k
        sk, ak, tk, lk = tidx[mk], in_a[mk], tile_of[mk], lane_of[mk]
        for t in range(T):
            mt = tk == t
            s_t, a_t, l_t = sk[mt], ak[mt], lk[mt]
            order = np.argsort(s_t, kind="stable")
            s_t, a_t, l_t = s_t[order], a_t[order], l_t[order]
            tiles_a[k][t] = (s_t[a_t], l_t[a_t])
            tiles_b[k][t] = (s_t[~a_t], l_t[~a_t])

    cdiv = lambda a, b: -(-a // b)
    CH_A = max(max(cdiv(len(tiles_a[k][t][0]), P), 1) for k in range(NCORES) for t in range(T))
    CH_B = max(cdiv(len(tiles_b[k][t][0]), P) for k in range(NCORES) for t in range(T))

    # group tiles in pairs; chunk sequence per group: A-run (t0 A-chunks, t1
    # A-chunks) then B-run.  Blocks of <=8 chunks per dma_gather instruction.
    groups = [tuple(range(g, min(g + 2, T))) for g in range(0, T, 2)]
    K_CH = CH_A + CH_B
    NCHUNK = T * K_CH
    E_pad = NCHUNK * P

    # compile-time metadata shared by all cores
    chunk_meta = []   # per chunk: (tile, first, last)
    blocks = []       # flat list per dma_gather: (win, chunk0, nchunks)
    grp_meta = []     # per group: dict(c0, nch, runs=[(win, c0, nch, blocks)])
    counts = {t: 0 for t in range(T)}
    total = {t: (CH_A + CH_B) for t in range(T)}
    gc = 0
    for grp in groups:
        gm = dict(grp=grp, c0=gc, runs=[])
        for win, chw in (("A", CH_A), ("B", CH_B)):
            if chw == 0:
                continue
            nch = chw * len(grp)
            rblocks = []
            for b0 in range(0, nch, 8):
                blk = (win, gc + b0, min(8, nch - b0))
                rblocks.append(blk)
                blocks.append(blk)
            gm["runs"].append((win, gc, nch, rblocks))
            for t in grp:
                for _ in range(chw):
                    c = counts[t]
                    chunk_meta.append((t, c == 0, c == total[t] - 1))
                    counts[t] += 1
                    gc += 1
        gm["nch"] = gc - gm["c0"]
        grp_meta.append(gm)
    assert gc == NCHUNK

    # per-core arrays
    import ml_dtypes
    bf = ml_dtypes.bfloat16
    idx_cols = E_pad // 16
    idx16 = np.zeros((NCORES, P, idx_cols), np.int16)
    dstrep = np.full((NCORES, P, E_pad), PAD_REP, np.uint8)
    selin = np.zeros((NCORES, P, E_pad), bf)

    for k in range(NCORES):
        flat_idx = np.zeros(E_pad, np.int16)
        flat_lane = np.full(E_pad, -1, np.int64)
        gc = 0
        for grp in groups:
            for win, chw in (("A", CH_A), ("B", CH_B)):
                if chw == 0:
                    continue
                for t in grp:
                    s_t, l_t = (tiles_a if win == "A" else tiles_b)[k][t]
                    n = len(s_t)
                    o = gc * P
                    flat_idx[o:o + n] = s_t.astype(np.int16)
                    flat_lane[o:o + n] = l_t
                    gc += chw
        # wrapped+replicated index layout per gather block
        for win, c0, nch in blocks:
            seg = flat_idx[c0 * P:(c0 + nch) * P]
            wrapped = seg.reshape(-1, 16).T            # [16, n/16]
            col0 = c0 * P // 16
            idx16[k, :, col0:col0 + wrapped.shape[1]] = np.tile(wrapped, (8, 1))
        rep = np.where(flat_lane >= 0, flat_lane, PAD_REP).astype(np.uint8)
        dstrep[k] = np.tile(rep[None, :], (P, 1))
        # host-built one-hot scatter matrices: sel[p, gc*P + m] = 1 iff edge
        # (gc, p) targets dst lane m (padding lanes stay all-zero)
        ee = np.nonzero(flat_lane >= 0)[0]
        p_of = ee % P
        col = (ee // P) * P + flat_lane[ee]
        selin[k][p_of, col] = 1

    pl = Plan()
    pl.N, pl.NL, pl.T = N, NL, T
    pl.G_B, pl.NA, pl.NB = G_B, NCORES * G_A, NCORES * G_B
    pl.TA = G_A // P
    pl.CH_A, pl.CH_B, pl.K_CH = CH_A, CH_B, K_CH
    pl.NCHUNK, pl.E_pad = NCHUNK, E_pad
    pl.groups, pl.chunk_meta, pl.blocks = groups, chunk_meta, blocks
    pl.grp_meta = grp_meta
    pl.idx16, pl.dstrep, pl.selin = idx16, dstrep, selin
    return pl


def _fold_bn(g, be, rm, rv, b, eps=1e-5):
    k = (g / np.sqrt(rv + eps)).astype(np.float64)
    c = (b.astype(np.float64) - rm) * k + be
    return k.astype(np.float32), c.astype(np.float32)


def _prep_weights(W, a_s, a_d, bias, g, be, rm, rv, bf16):
    """Host precompute: [Wmain | Wsd] blocks and folded BN constants."""
    IN = W.shape[0]
    Hh, C = a_s.shape
    Wmain = W.astype(np.float64)
    Ws = np.zeros((IN, Hh), np.float64)
    Wd = np.zeros((IN, Hh), np.float64)
    for h in range(Hh):
        blk = W[:, h * C:(h + 1) * C].astype(np.float64)
        Ws[:, h] = blk @ a_s[h].astype(np.float64)
        Wd[:, h] = blk @ a_d[h].astype(np.float64)
    Wsd = np.concatenate([Ws, Wd], axis=1)            # [IN, 2H]
    k, c = _fold_bn(np.asarray(g, np.float64), np.asarray(be, np.float64),
                    np.asarray(rm, np.float64), np.asarray(rv, np.float64),
                    np.asarray(bias, np.float64))
    # fold the BN scale into the message weights: the aggregated message is
    # (sum w h)/(sum w), so scaling h's columns by k is exact
    Wmain = Wmain * k[None, :].astype(np.float64)
    return Wmain.astype(bf16), Wsd.astype(bf16), np.tile(c, (P, 1))


# ----------------------------------------------------------------------------
# device program
# ----------------------------------------------------------------------------

def _build_program(pl, dims):
    import concourse.tile as tile
    from concourse import bacc, mybir

    f32 = mybir.dt.float32
    bf16 = mybir.dt.bfloat16
    i16 = mybir.dt.int16
    u8 = mybir.dt.uint8

    NL, T = pl.NL, pl.T
    layers = dims["layers"]   # list of dicts: IN, H, C, ROWW, AUGW
    HID = dims["HID"]

    nc = bacc.Bacc("TRN2", target_bir_lowering=False, debug=False,
                   num_devices=NCORES)

    def din(name, shape, dt=f32):
        return nc.dram_tensor(name, list(shape), dt, kind="ExternalInput").ap()

    x_fm = din("x_fm", (layers[0]["IN"], NL), bf16)
    eidx = din("eidx", pl.idx16.shape[1:], i16)
    dstrep_d = din("dstrep", pl.dstrep.shape[1:], u8)
    selin_d = din("selin", pl.selin.shape[1:], bf16)
    iota_col_d = din("iota_col", (P, 1))
    Wmain_d, Wsd_d, crep_d = [], [], []
    for li, L in enumerate(layers):
        Wmain_d.append(din(f"Wmain{li}", (L["IN"], L["H"] * L["C"]), bf16))
        Wsd_d.append(din(f"Wsd{li}", (L["IN"], 2 * L["H"]), bf16))
        crep_d.append(din(f"crep{li}", (P, L["H"] * L["C"] if L["concat"] else L["C"])))
    Wc_d = din("Wc", (HID, 2), bf16)
    bcrep_d = din("bcrep", (P, 2))

    out_d = nc.dram_tensor("out", [NL, 2], f32, kind="ExternalOutput").ap()

    # internal DRAM
    haug_locA, haug_locB, haug_fullA, haug_fullB = [], [], [], []
    for li, L in enumerate(layers):
        haug_locA.append(nc.dram_tensor(f"haug_locA{li}", [G_A, L["ROWW"]], bf16).ap())
        haug_locB.append(nc.dram_tensor(f"haug_locB{li}", [pl.G_B, L["ROWW"]], bf16).ap())
        haug_fullA.append(nc.dram_tensor(f"haug_fullA{li}", [pl.NA, L["ROWW"]], bf16,
                                         addr_space="Shared").ap())
        haug_fullB.append(nc.dram_tensor(f"haug_fullB{li}", [pl.NB, L["ROWW"]], bf16,
                                         addr_space="Shared").ap())

    with tile.TileContext(nc) as tc:
        _emit(tc, nc, pl, dims, locals(), mybir)
    nc.compile()
    return nc


def _emit(tc, nc, pl, dims, refs, mybir):
    from contextlib import ExitStack
    from concourse.masks import make_identity

    f32 = mybir.dt.float32
    bf16 = mybir.dt.bfloat16
    i16 = mybir.dt.int16
    u8 = mybir.dt.uint8
    AF = mybir.ActivationFunctionType
    OP = mybir.AluOpType

    NL, T, N = pl.NL, pl.T, pl.N
    layers = dims["layers"]
    HID = dims["HID"]
    x_fm, eidx, dstrep_d = refs["x_fm"], refs["eidx"], refs["dstrep_d"]
    selin_d, iota_col_d = refs["selin_d"], refs["iota_col_d"]
    Wmain_d, Wsd_d, crep_d = refs["Wmain_d"], refs["Wsd_d"], refs["crep_d"]
    Wc_d, bcrep_d, out_d = refs["Wc_d"], refs["bcrep_d"], refs["out_d"]
    haug_locA, haug_locB = refs["haug_locA"], refs["haug_locB"]
    haug_fullA, haug_fullB = refs["haug_fullA"], refs["haug_fullB"]

    ctx = ExitStack()
    with ctx:
        const = ctx.enter_context(tc.tile_pool(name="const", bufs=1))
        wpool = ctx.enter_context(tc.tile_pool(name="wpool", bufs=1))
        mm_in = ctx.enter_context(tc.tile_pool(name="mm_in", bufs=3))
        aug_pool = ctx.enter_context(tc.tile_pool(name="aug", bufs=3))
        gpool = ctx.enter_context(tc.tile_pool(name="gpool", bufs=3))
        rep_pool = ctx.enter_context(tc.tile_pool(name="rep", bufs=2))
        sel_pool = ctx.enter_context(tc.tile_pool(name="sel", bufs=6))
        wg_pool = ctx.enter_context(tc.tile_pool(name="wg", bufs=4))
        ev_pool = ctx.enter_context(tc.tile_pool(name="ev", bufs=2))
        post_pool = ctx.enter_context(tc.tile_pool(name="post", bufs=3))
        zc_pool = ctx.enter_context(tc.tile_pool(name="zcp", bufs=12))
        keep = ctx.enter_context(tc.tile_pool(name="keep", bufs=1))

        # ---- resident constants
        iota_col = const.tile([P, 1], f32)
        nc.sync.dma_start(out=iota_col[:], in_=iota_col_d[:])
        ident_f = const.tile([P, P], f32)
        make_identity(nc, ident_f[:])
        ident = const.tile([P, P], bf16)
        nc.vector.tensor_copy(ident[:], ident_f[:])
        idx_sb = const.tile(list(pl.idx16.shape[1:]), i16)
        nc.sync.dma_start(out=idx_sb[:], in_=eidx[:])

        Wmain_sb, Wsd_sb, crep_sb = [], [], []
        for li, L in enumerate(layers):
            wm = wpool.tile([P, L["IN"] // P, L["H"] * L["C"]], bf16, tag=f"wm{li}")
            nc.gpsimd.dma_start(
                out=wm[:],
                in_=Wmain_d[li][:].rearrange("(a p) n -> p a n", p=P))
            Wmain_sb.append(wm)
            ws = wpool.tile([P, L["IN"] // P, 2 * L["H"]], bf16, tag=f"ws{li}")
            nc.gpsimd.dma_start(
                out=ws[:],
                in_=Wsd_d[li][:].rearrange("(a p) n -> p a n", p=P))
            Wsd_sb.append(ws)
            FW = L["H"] * L["C"] if L["concat"] else L["C"]
            ct = wpool.tile([P, FW], f32, tag=f"c{li}")
            nc.sync.dma_start(out=ct[:], in_=crep_d[li][:])
            crep_sb.append(ct)
        Wc_sb = wpool.tile([P, 2], bf16)
        nc.gpsimd.dma_start(out=Wc_sb[:], in_=Wc_d[:])
        bcrep_sb = wpool.tile([P, 2], f32)
        nc.sync.dma_start(out=bcrep_sb[:], in_=bcrep_d[:])

        d_loc = [keep.tile([P, T * L["H"]], bf16, tag=f"dloc{li}",
                           name=f"dloc{li}")
                 for li, L in enumerate(layers)]
        for dl in d_loc:
            nc.vector.memset(dl[:], 0.0)

        def rows_of(t):
            return min(P, NL - t * P)

        def allgather(li, win):
            loc = (haug_locA if win == "A" else haug_locB)[li]
            full = (haug_fullA if win == "A" else haug_fullB)[li]
            if dims.get("nocc"):
                n = G_A if win == "A" else pl.G_B
                nc.sync.dma_start(out=full[:n, :], in_=loc[:])
            else:
                nc.gpsimd.collective_compute(
                    "AllGather", mybir.AluOpType.bypass,
                    replica_groups=[list(range(NCORES))],
                    ins=[loc[:].opt()],
                    outs=[full[:].opt()],
                )

        # ------------------------------------------------------------------
        def mm_tile(li, t, mm_ps, mm_sd_ps, lhs_tiles=None):
            """One row-tile of the h = z @ W phase for layer li.  lhs_tiles
            (the previous agg's transposed zc SBUF tiles) feed the PE
            directly when given; layer 0 loads x from DRAM instead."""
            L = layers[li]
            H, C, IN, ROWW = L["H"], L["C"], L["IN"], L["ROWW"]
            NF = H * C
            KT = IN // P
            mt = rows_of(t)
            if lhs_tiles is None:
                lhs = mm_in.tile([P, KT, P], bf16, tag="lhs")
                nc.gpsimd.dma_start(
                    out=lhs[:, :, :mt],
                    in_=x_fm[:].rearrange("(a p) n -> p a n", p=P)[:, :, t * P:t * P + mt])
                blocks = [lhs[:, kk, :mt] for kk in range(KT)]
            else:
                assert len(lhs_tiles) == KT
                blocks = [zc[:, :mt] for zc in lhs_tiles]
            ps1 = mm_ps.tile([P, NF], f32)
            ps2 = mm_sd_ps.tile([P, 2 * H], f32)
            for kk in range(KT):
                nc.tensor.matmul(out=ps1[:mt, :], lhsT=blocks[kk],
                                 rhs=Wmain_sb[li][:, kk, :],
                                 start=(kk == 0), stop=(kk == KT - 1))
                nc.tensor.matmul(out=ps2[:mt, :], lhsT=blocks[kk],
                                 rhs=Wsd_sb[li][:, kk, :],
                                 start=(kk == 0), stop=(kk == KT - 1))
            aug = aug_pool.tile([P, ROWW], bf16, tag="aug")
            a3 = aug[:, :H * (C + 1)].rearrange("p (h c) -> p h c", h=H)
            nc.scalar.activation(
                out=a3[:mt, :, :C],
                in_=ps1[:mt, :].rearrange("p (h c) -> p h c", h=H),
                func=AF.Copy)
            nc.vector.memset(a3[:mt, :, C:C + 1], 1.0)
            nc.vector.tensor_copy(out=aug[:mt, H * (C + 1):H * (C + 2)],
                                  in_=ps2[:mt, :H])
            nc.vector.memset(aug[:mt, L["AUGW"]:], 0.0)
            nc.vector.tensor_copy(
                out=d_loc[li][:mt, t * H:(t + 1) * H],
                in_=ps2[:mt, H:2 * H])
            if t < pl.TA:
                nc.sync.dma_start(out=haug_locA[li][t * P:t * P + mt, :],
                                  in_=aug[:mt, :])
                if t == pl.TA - 1:
                    allgather(li, "A")
            else:
                o = t * P - G_A
                nc.sync.dma_start(out=haug_locB[li][o:o + mt, :],
                                  in_=aug[:mt, :])
                if t == T - 1:
                    allgather(li, "B")

        # ------------------------------------------------------------------
        def agg_phase(li, agg_ps, den_ps, dexp_ps, tr_ps, next_mm=None):
            L = layers[li]
            H, C, ROWW = L["H"], L["C"], L["ROWW"]
            CP1 = C + 1

            def alloc_group(gm):
                grp = gm["grp"]
                st = dict(gm=gm)
                st["ps_main"] = {
                    t: agg_ps.tile([P, H * C + (4 if H == 1 else 0)],
                                   f32, tag="agm", name=f"agm{t}")
                    for t in grp}
                if H > 1:
                    st["ps_den"] = {t: den_ps.tile([P, H], f32, tag="den",
                                                   name=f"den{t}")[:]
                                    for t in grp}
                g_c0, g_nch = gm["c0"], gm["nch"]
                rep_sb = rep_pool.tile([P, g_nch * P], u8, tag="rep")
                nc.sync.dma_start(out=rep_sb[:],
                                  in_=dstrep_d[:, g_c0 * P:(g_c0 + g_nch) * P])
                st["rep_sb"] = rep_sb
                return st

            def do_run(st, widx):
                gm, ps_main = st["gm"], st["ps_main"]
                ps_den = st.get("ps_den")
                rep_sb = st["rep_sb"]
                g_c0 = gm["c0"]
                if widx >= len(gm["runs"]):
                    return
                win, c0, nch, rblocks = gm["runs"][widx]
                psd = dexp_ps.tile([P, nch * H], f32, tag="dexp",
                                   name="psd")[:]
                if True:
                    table = (haug_fullA if win == "A" else haug_fullB)[li]
                    # host-precomputed one-hot scatter matrices for the run
                    selr = sel_pool.tile([P, nch * P], bf16, tag="sel")
                    nc.gpsimd.dma_start(
                        out=selr[:], in_=selin_d[:, c0 * P:(c0 + nch) * P])
                    # gathers for this run
                    gtiles = []
                    for bwin, bc0, bn in rblocks:
                        gt = gpool.tile([P, bn, ROWW], bf16, tag="G")
                        nc.gpsimd.dma_gather(
                            out_ap=gt[:],
                            in_ap=table[:],
                            idxs_ap=idx_sb[:, bc0 * P // 16:(bc0 + bn) * P // 16],
                            num_idxs=bn * P, num_idxs_reg=bn * P,
                            elem_size=ROWW)
                        gtiles.append((bc0, bn, gt))
                    # d_exp for the run: batch-build all selT one-hots of a
                    # gather block in ONE is_equal, then one tiny matmul per
                    # chunk gathers d[dst] into per-edge lanes
                    for (bc0, bn, gt) in [(b, n, None) for _, b, n in rblocks]:
                        selT = sel_pool.tile([P, bn * P], bf16, tag="selT")
                        o = (bc0 - g_c0) * P
                        nc.vector.tensor_scalar(
                            out=selT[:], in0=rep_sb[:, o:o + bn * P],
                            scalar1=iota_col[:], scalar2=None, op0=OP.is_equal)
                        for j in range(bn):
                            gc = bc0 + j
                            ci = gc - c0
                            t = pl.chunk_meta[gc][0]
                            nc.tensor.matmul(out=psd[:, ci * H:(ci + 1) * H],
                                             lhsT=selT[:, j * P:(j + 1) * P],
                                             rhs=d_loc[li][:, t * H:(t + 1) * H],
                                             start=True, stop=True)
                    # batched e-values for the run
                    ev = ev_pool.tile([P, nch * H], f32, tag="ev")
                    sv = ev_pool.tile([P, nch * H], f32, tag="sv")
                    for (bc0, bn, gt) in gtiles:
                        nc.vector.tensor_copy(
                            out=sv[:, (bc0 - c0) * H:(bc0 - c0 + bn) * H]
                                .rearrange("p (b h) -> p b h", h=H),
                            in_=gt[:, :, H * CP1:H * CP1 + H])
                    nc.vector.tensor_add(ev[:], sv[:], psd)
                    nc.vector.scalar_tensor_tensor(
                        out=ev[:], in0=ev[:], scalar=0.2, op0=OP.mult,
                        op1=OP.max, in1=ev[:])
                    nc.vector.tensor_scalar(out=ev[:], in0=ev[:], scalar1=ECLAMP,
                                            scalar2=None, op0=OP.min)
                    nc.scalar.activation(out=ev[:], in_=ev[:], func=AF.Exp)
                    evb = ev_pool.tile([P, nch * H], bf16, tag="evb")
                    nc.vector.tensor_copy(evb[:], ev[:])
                    # weighted scatter matmuls; wg for a whole gather block is
                    # computed with a single broadcast multiply
                    CW = CP1 if H > 1 else CP1 + 3
                    for (bc0, bn, gt) in gtiles:
                        wgb = wg_pool.tile([P, bn, H, CW], bf16, tag="wg")
                        nc.vector.tensor_tensor(
                            out=wgb[:],
                            in0=gt[:, :, :H * CW].rearrange("p b (h c) -> p b h c", h=H),
                            in1=evb[:, (bc0 - c0) * H:(bc0 - c0 + bn) * H]
                                .rearrange("p (b h c) -> p b h c", c=1, h=H)
                                .to_broadcast([P, bn, H, CW]),
                            op=OP.mult)
                        for j in range(bn):
                            gc = bc0 + j
                            ci = gc - c0
                            t, first, last = pl.chunk_meta[gc]
                            sel = selr[:, ci * P:(ci + 1) * P]
                            wg = wgb[:, j]
                            if H > 1:
                                nc.tensor.matmul(
                                    out=ps_main[t][:].rearrange("p (h c) -> p h c", h=H),
                                    lhsT=sel, rhs=wg[:, :, :C],
                                    start=first, stop=last)
                                nc.tensor.matmul(
                                    out=ps_den[t], lhsT=sel,
                                    rhs=wg[:, :, C:CP1].rearrange("p h c -> p (h c)"),
                                    start=first, stop=last)
                            else:
                                nc.tensor.matmul(
                                    out=ps_main[t][:], lhsT=sel,
                                    rhs=wg[:, 0, :],
                                    start=first, stop=last)

            def post_group(st):
                gm, ps_main = st["gm"], st["ps_main"]
                ps_den = st.get("ps_den")
                for t in gm["grp"]:
                    mt = rows_of(t)
                    FW = H * C if L["concat"] else C
                    rc = post_pool.tile([P, H], f32, tag="rc")
                    if H > 1:
                        nc.vector.reciprocal(rc[:], ps_den[t])
                    else:
                        nc.vector.reciprocal(rc[:], ps_main[t][:, C:C + 1])
                    # per-head alpha normalization on the ACT engine (the BN
                    # scale k is host-folded into Wmain)
                    zt = post_pool.tile([P, FW], f32, tag="zt")
                    for h in range(H):
                        nc.scalar.activation(
                            out=zt[:, h * C:(h + 1) * C],
                            in_=ps_main[t][:, h * C:(h + 1) * C],
                            func=AF.Copy, scale=rc[:, h:h + 1])
                    nc.vector.tensor_tensor(out=zt[:], in0=zt[:], in1=crep_sb[li][:],
                                            op=OP.add)
                    # ELU(x) = max(x, exp(-relu(-x)) - 1), inner steps on ACT
                    mneg = post_pool.tile([P, FW], f32, tag="mneg")
                    nc.scalar.activation(out=mneg[:], in_=zt[:], func=AF.Relu,
                                         scale=-1.0)
                    nc.scalar.activation(out=mneg[:], in_=mneg[:], func=AF.Exp,
                                         scale=-1.0)
                    zf = post_pool.tile([P, FW], bf16, tag="zf")
                    nc.vector.scalar_tensor_tensor(
                        out=zf[:], in0=mneg[:], scalar=-1.0,
                        op0=OP.add, op1=OP.max, in1=zt[:])
                    if li + 1 < len(layers):
                        # transpose to feature-major; the zc SBUF tiles feed
                        # the next layer's matmul directly (no DRAM bounce)
                        tiles = []
                        for h in range(FW // P):
                            pt = tr_ps.tile([P, P], bf16, tag="tr")
                            nc.tensor.matmul(out=pt[:], lhsT=zf[:, h * P:(h + 1) * P],
                                             rhs=ident[:], is_transpose=True,
                                             start=True, stop=True)
                            zc = zc_pool.tile([P, P], bf16, tag="zc")
                            nc.scalar.activation(out=zc[:], in_=pt[:], func=AF.Copy)
                            tiles.append(zc)
                        if next_mm is not None:
                            next_mm(t, tiles)
                    else:
                        # classifier
                        pt = tr_ps.tile([P, P], bf16, tag="tr")
                        nc.tensor.matmul(out=pt[:], lhsT=zf[:, :P], rhs=ident[:],
                                         is_transpose=True, start=True, stop=True)
                        zc = post_pool.tile([P, P], bf16, tag="zcr")
                        nc.scalar.activation(out=zc[:], in_=pt[:], func=AF.Copy)
                        pc = den_ps.tile([P, 2], f32, tag="pc")
                        nc.tensor.matmul(out=pc[:mt, :], lhsT=zc[:, :mt], rhs=Wc_sb[:],
                                         start=True, stop=True)
                        ot = post_pool.tile([P, 2], f32, tag="ot")
                        nc.vector.tensor_tensor(out=ot[:mt, :], in0=pc[:mt, :],
                                                in1=bcrep_sb[:mt, :], op=OP.add)
                        nc.sync.dma_start(out=out_d[t * P:t * P + mt, :],
                                          in_=ot[:mt, :])

            # driver: per group A-run, B-run, post (which hands zc tiles to
            # the fused next-layer matmul via next_mm)
            for gm in pl.grp_meta:
                st = alloc_group(gm)
                do_run(st, 0)
                do_run(st, 1)
                post_group(st)

        for _rep in range(dims.get("reps", 1)):
            # layer 0 matmul phase standalone (input x comes from DRAM)
            with tc.tile_pool(name="mm_ps0", bufs=2, space="PSUM") as mm_ps, \
                 tc.tile_pool(name="mm_sd_ps0", bufs=2, space="PSUM") as mm_sd_ps:
                for t in range(T):
                    mm_tile(0, t, mm_ps, mm_sd_ps)
            # each agg phase interleaves the NEXT layer's matmul tiles so its
            # AllGathers issue (and largely complete) before this agg ends
            for li in range(len(layers)):
                if li + 1 < len(layers):
                    with tc.tile_pool(name=f"agg_ps{li}", bufs=2, space="PSUM") as agg_ps, \
                         tc.tile_pool(name=f"den_ps{li}", bufs=2, space="PSUM") as den_ps, \
                         tc.tile_pool(name=f"dexp_ps{li}", bufs=1, space="PSUM") as dexp_ps, \
                         tc.tile_pool(name=f"tr_ps{li}", bufs=1, space="PSUM") as tr_ps, \
                         tc.tile_pool(name=f"mm_ps{li + 1}", bufs=1, space="PSUM") as nmm_ps, \
                         tc.tile_pool(name=f"mm_sd_ps{li + 1}", bufs=1, space="PSUM") as nmm_sd_ps:
                        def next_mm(t, tiles, _li=li + 1, _a=nmm_ps, _b=nmm_sd_ps):
                            mm_tile(_li, t, _a, _b, lhs_tiles=tiles)
                        agg_phase(li, agg_ps, den_ps, dexp_ps, tr_ps, next_mm)
                else:
                    with tc.tile_pool(name=f"agg_ps{li}", bufs=2, space="PSUM") as agg_ps, \
                         tc.tile_pool(name=f"den_ps{li}", bufs=2, space="PSUM") as den_ps, \
                         tc.tile_pool(name=f"dexp_ps{li}", bufs=1, space="PSUM") as dexp_ps, \
                         tc.tile_pool(name=f"tr_ps{li}", bufs=2, space="PSUM") as tr_ps:
                        agg_phase(li, agg_ps, den_ps, dexp_ps, tr_ps, None)


# ----------------------------------------------------------------------------
# entry point
# ----------------------------------------------------------------------------

def _layer_dims(IN, H, C, concat):
    used = H * (C + 1) + H           # features+ones | s columns
    roww = -(-used * 2 // 256) * 128  # pad row to multiple of 256 bytes (bf16)
    return dict(IN=IN, H=H, C=C, concat=concat, ROWW=roww, AUGW=used)


def build_all(x, edge_index, W1, a1s, a1d, b1, g1, be1, rm1, rv1,
              W2, a2s, a2d, b2, g2, be2, rm2, rv2,
              W3, a3s, a3d, b3, g3, be3, rm3, rv3, Wc, bc,
              nocc=False, reps=1, gather0=False):
    import ml_dtypes
    bf16 = ml_dtypes.bfloat16
    x = np.asarray(x)
    N, IN = x.shape
    HID = W3.shape[1]
    H = a1s.shape[0]
    pl = _plan_edges(N, np.asarray(edge_index))
    if gather0:
        pl.idx16 = np.zeros_like(pl.idx16)
    layers = [
        _layer_dims(IN, H, W1.shape[1] // H, True),
        _layer_dims(W1.shape[1], H, W2.shape[1] // H, True),
        _layer_dims(W2.shape[1], 1, W3.shape[1], False),
    ]
    dims = dict(layers=layers, HID=HID, nocc=nocc, reps=reps)

    Wm1, Wsd1, c1 = _prep_weights(W1, a1s, a1d, b1, g1, be1, rm1, rv1, bf16)
    Wm2, Wsd2, c2 = _prep_weights(W2, a2s, a2d, b2, g2, be2, rm2, rv2, bf16)
    Wm3, Wsd3, c3 = _prep_weights(W3, a3s, a3d, b3, g3, be3, rm3, rv3, bf16)

    iota_col = np.arange(P, dtype=np.float32).reshape(P, 1)

    in_maps = []
    for k in range(NCORES):
        m = dict(
            x_fm=np.ascontiguousarray(x[k * pl.NL:(k + 1) * pl.NL].T).astype(bf16),
            eidx=pl.idx16[k], dstrep=pl.dstrep[k], selin=pl.selin[k],
            iota_col=iota_col,
            Wmain0=Wm1, Wsd0=Wsd1, crep0=c1,
            Wmain1=Wm2, Wsd1=Wsd2, crep1=c2,
            Wmain2=Wm3, Wsd2=Wsd3, crep2=c3,
            Wc=np.asarray(Wc, np.float32).astype(bf16),
            bcrep=np.tile(np.asarray(bc, np.float32), (P, 1)),
        )
        in_maps.append(m)

    nc = _build_program(pl, dims)
    return nc, in_maps, pl


def kernel(**inputs):
    from concourse.bass_utils import run_bass_kernel_spmd
    nc, in_maps, pl = build_all(**inputs)
    res = run_bass_kernel_spmd(nc, in_maps, core_ids=list(range(NCORES)))
    out = np.concatenate([res.results[k]["out"] for k in range(NCORES)], axis=0)
    return out.astype(np.float32)
